# revision 7
# baseline (speedup 1.0000x reference)
"""Trainium2 Bass kernel for nn_AttentionReweightingFusion.

Contract: kernel(**inputs) takes FULL (unsharded) numpy inputs as produced by
setup_inputs() and returns the FULL [16384, 1024] float32 output.

Strategy (pure data parallel over 8 NeuronCores, weights replicated):
  - 2048 batch rows per core, processed in 4 tiles of 512 rows.
  - Features and big weights are pre-cast to bf16 on the host, halving HBM
    input traffic; wv@wo is collapsed host-side into a single wc matrix.
  - Per-row scalar math (missing-type coefficients, thresholds) runs in exact
    fp32 in row-major layout where per-row values are native [128,1] scalars.
  - Row-major -> feature-major conversion of the combined features uses the
    xbar DMA transpose (one call turns a [128 rows, 512 feat] chunk into the
    [feat, chunk, row] layout the matmuls consume), keeping the PE stream
    free for real matmuls.
  - The difficulty column's rank-1 contribution to the first dc matmul is
    computed as a normal K=128 matmul against a broadcast difficulty tile
    using a host-prepared weight spread over 128 partitions.
  - seq_len==kv_len==1 MHA reduces to out_proj(v_proj(x)) == x @ wc.
"""

import os

import numpy as np

H = 512
B_FULL = 16384
N_CORES = 8
B_CORE = B_FULL // N_CORES          # 2048
TILE_N = 512                        # batch rows per compute tile
N_TILES = B_CORE // TILE_N          # 4
PC = H // 128                       # feature chunks of 128 (4)
RC_TOT = B_CORE // 128              # row chunks per core (16)

_CACHE: dict = {}

# Exposed for test.py after a profiled run
last_exec_time_ns = None
last_trace_path = None
last_scope_times = None


def _build_program(use_bvo=False):
    from contextlib import ExitStack

    import concourse.bacc as bacc
    import concourse.mybir as mybir
    import concourse.tile as tile
    from concourse.masks import make_identity

    dt = mybir.dt
    f32 = dt.float32
    bf16 = dt.bfloat16
    AF = mybir.ActivationFunctionType
    OP = mybir.AluOpType

    nc = bacc.Bacc(num_swdge_queues=4)

    # ---------------- DRAM I/O (per-core shapes) ----------------
    d_img = nc.dram_tensor("image_feat", [B_CORE, H], bf16, kind="ExternalInput")
    d_txt = nc.dram_tensor("text_feat", [B_CORE, H], bf16, kind="ExternalInput")
    d_eimg = nc.dram_tensor("enhanced_image_feat", [B_CORE, H], bf16, kind="ExternalInput")
    d_etxt = nc.dram_tensor("enhanced_text_feat", [B_CORE, H], bf16, kind="ExternalInput")
    d_qual = nc.dram_tensor("quality", [B_CORE, 11], f32, kind="ExternalInput")
    d_miss = nc.dram_tensor("missing_f", [B_CORE], f32, kind="ExternalInput")

    d_dcw1 = nc.dram_tensor("dcw1b", [H, H], bf16, kind="ExternalInput")
    d_wr1 = nc.dram_tensor("wr1b", [128, H], bf16, kind="ExternalInput")
    d_dcb1 = nc.dram_tensor("dc_b1", [H], f32, kind="ExternalInput")
    d_dcw2 = nc.dram_tensor("dcw2b", [H, H], bf16, kind="ExternalInput")
    d_dcb2h = nc.dram_tensor("dcb2h", [H], f32, kind="ExternalInput")
    d_wc = nc.dram_tensor("wcb", [H, H], bf16, kind="ExternalInput")

    d_qa_w1 = nc.dram_tensor("qa_w1", [11, 64], bf16, kind="ExternalInput")
    d_qa_b1 = nc.dram_tensor("qa_b1", [64], f32, kind="ExternalInput")
    d_qa_w2 = nc.dram_tensor("qa_w2", [64, 32], bf16, kind="ExternalInput")
    d_qa_b2 = nc.dram_tensor("qa_b2", [32], f32, kind="ExternalInput")
    d_qa_w3 = nc.dram_tensor("qa_w3", [32, 1], bf16, kind="ExternalInput")
    d_qab3h = nc.dram_tensor("qab3h", [1], f32, kind="ExternalInput")
    d_miw1p = nc.dram_tensor("miw1p", [11, 32], bf16, kind="ExternalInput")
    d_mi_b1 = nc.dram_tensor("mi_b1", [32], f32, kind="ExternalInput")
    d_wdiff = nc.dram_tensor("wdiff", [32, 1], bf16, kind="ExternalInput")
    d_db = nc.dram_tensor("db", [1], f32, kind="ExternalInput")
    if use_bvo:
        d_bvo = nc.dram_tensor("bvo", [H], bf16, kind="ExternalInput")

    d_out = nc.dram_tensor("out", [B_CORE, 2 * H], f32, kind="ExternalOutput")

    with tile.TileContext(nc) as tc, ExitStack() as ctx:
        singles = ctx.enter_context(tc.tile_pool(name="singles", bufs=1))
        inp = ctx.enter_context(tc.tile_pool(name="inp", bufs=6))
        ps_mm = ctx.enter_context(tc.tile_pool(name="ps_mm", bufs=6, space="PSUM"))
        ps_tr = ctx.enter_context(tc.tile_pool(name="ps_tr", bufs=2, space="PSUM"))
        finp = ctx.enter_context(tc.tile_pool(name="finp", bufs=12))
        fintp = ctx.enter_context(tc.tile_pool(name="fintp", bufs=4))
        g1p = ctx.enter_context(tc.tile_pool(name="g1p", bufs=10))
        stp = ctx.enter_context(tc.tile_pool(name="stp", bufs=9))
        t1p = ctx.enter_context(tc.tile_pool(name="t1p", bufs=3))
        compp = ctx.enter_context(tc.tile_pool(name="compp", bufs=9))
        outp = ctx.enter_context(tc.tile_pool(name="outp", bufs=6))

        feats = [d_img, d_txt, d_eimg, d_etxt]

        # ---------------- async DMA loads ----------------
        qual = singles.tile([128, RC_TOT, 11], f32, tag="qual")
        nc.sync.dma_start(out=qual, in_=d_qual.rearrange("(c p) f -> p c f", p=128))
        mrm = singles.tile([128, RC_TOT], f32, tag="mrm")
        nc.sync.dma_start(out=mrm, in_=d_miss.rearrange("(c p) -> p c", p=128))
        # difficulty column as a [1, B] row (strided DRAM gather)
        dT_raw = singles.tile([1, B_CORE], f32, tag="dT_raw")
        nc.sync.dma_start(out=dT_raw, in_=d_qual[:, 10:11].rearrange("b 1 -> 1 b"))

        dcw1 = singles.tile([128, PC, H], bf16, tag="dcw1")
        nc.sync.dma_start(out=dcw1, in_=d_dcw1.rearrange("(c p) f -> p c f", p=128))
        wr1 = singles.tile([128, H], bf16, tag="wr1")
        nc.sync.dma_start(out=wr1, in_=d_wr1[:, :])
        dcw2 = singles.tile([128, PC, H], bf16, tag="dcw2")
        nc.sync.dma_start(out=dcw2, in_=d_dcw2.rearrange("(c p) f -> p c f", p=128))
        wc = singles.tile([128, PC, H], bf16, tag="wc")
        nc.sync.dma_start(out=wc, in_=d_wc.rearrange("(c p) f -> p c f", p=128))

        dcb1 = singles.tile([128, PC], f32, tag="dcb1")
        nc.sync.dma_start(out=dcb1, in_=d_dcb1.rearrange("(m p) -> p m", p=128))
        dcb2h = singles.tile([128, PC], f32, tag="dcb2h")
        nc.sync.dma_start(out=dcb2h, in_=d_dcb2h.rearrange("(m p) -> p m", p=128))

        qaw1 = singles.tile([11, 64], bf16, tag="qaw1")
        nc.sync.dma_start(out=qaw1, in_=d_qa_w1[:, :])
        qaw2 = singles.tile([64, 32], bf16, tag="qaw2")
        nc.sync.dma_start(out=qaw2, in_=d_qa_w2[:, :])
        qaw3 = singles.tile([32, 1], bf16, tag="qaw3")
        nc.sync.dma_start(out=qaw3, in_=d_qa_w3[:, :])
        miw1 = singles.tile([11, 32], bf16, tag="miw1")
        nc.sync.dma_start(out=miw1, in_=d_miw1p[:, :])
        wdiff = singles.tile([32, 1], bf16, tag="wdiff")
        nc.sync.dma_start(out=wdiff, in_=d_wdiff[:, :])
        qab1 = singles.tile([64, 1], f32, tag="qab1")
        nc.sync.dma_start(out=qab1, in_=d_qa_b1[:].unsqueeze(1))
        qab2 = singles.tile([32, 1], f32, tag="qab2")
        nc.sync.dma_start(out=qab2, in_=d_qa_b2[:].unsqueeze(1))
        qab3h = singles.tile([1, 1], f32, tag="qab3h")
        nc.sync.dma_start(out=qab3h, in_=d_qab3h[:].unsqueeze(1))
        mib1 = singles.tile([32, 1], f32, tag="mib1")
        nc.sync.dma_start(out=mib1, in_=d_mi_b1[:].unsqueeze(1))
        db = singles.tile([1, 1], f32, tag="db")
        nc.sync.dma_start(out=db, in_=d_db[:].unsqueeze(1))
        if use_bvo:
            bvo = singles.tile([1, H], bf16, tag="bvo")
            nc.sync.dma_start(out=bvo, in_=d_bvo[:].unsqueeze(0))
            ones_r = singles.tile([1, 128], bf16, tag="ones_r")
            nc.vector.memset(ones_r, 1.0)

        def emit_loads(t):
            in_sb = []
            for dten in feats:
                it = inp.tile([128, PC, H], bf16, tag="in", name="it")
                nc.sync.dma_start(
                    out=it,
                    in_=dten[t * TILE_N:(t + 1) * TILE_N, :].rearrange(
                        "(c p) f -> p c f", p=128))
                in_sb.append(it)
            return in_sb

        in_sb0 = emit_loads(0)

        ident = singles.tile([128, 128], bf16, tag="ident")
        make_identity(nc, ident)

        prol = nc.named_scope("prol")
        prol.__enter__()

        # broadcast difficulty/2 for the rank-1 z1 term and the t1 product
        Dball = singles.tile([128, B_CORE], f32, tag="Dball")
        nc.gpsimd.partition_broadcast(Dball, dT_raw)
        DballB = singles.tile([128, B_CORE], bf16, tag="DballB")   # D/2 bf16
        nc.gpsimd.tensor_scalar(DballB, Dball, 0.5, None, OP.mult)

        # ---------------- exact fp32 per-row coefficient math ----------------
        qual_bf = singles.tile([128, RC_TOT, 11], bf16, tag="qual_bf")
        nc.vector.tensor_copy(qual_bf, qual)

        def sc(tag):
            return singles.tile([128, RC_TOT], f32, tag=tag, name=tag)

        img_imp = qual[:, :, 6:7].rearrange("p c 1 -> p c")
        text_imp = qual[:, :, 7:8].rearrange("p c 1 -> p c")
        img_auth = qual[:, :, 8:9].rearrange("p c 1 -> p c")
        text_auth = qual[:, :, 9:10].rearrange("p c 1 -> p c")

        e0 = sc("e0"); e1 = sc("e1"); e2 = sc("e2")
        nc.vector.tensor_scalar(e0, mrm, 0.5, None, OP.is_lt)
        nc.vector.tensor_scalar(e1, mrm, 1.0, None, OP.is_equal)
        nc.vector.tensor_scalar(e2, mrm, 1.5, None, OP.is_gt)

        den = sc("den"); ratio = sc("ratio")
        nc.vector.scalar_tensor_tensor(den, img_imp, 1e-8, text_imp, OP.add, OP.add)
        nc.vector.reciprocal(den, den)
        nc.vector.tensor_mul(ratio, img_imp, den)
        ghi = sc("ghi"); glo = sc("glo"); si0 = sc("si0"); st0 = sc("st0")
        nc.vector.tensor_scalar(ghi, ratio, 0.6, None, OP.is_gt)
        nc.vector.tensor_scalar(glo, ratio, 0.4, None, OP.is_lt)
        nc.vector.tensor_sub(si0, ghi, glo)
        nc.vector.tensor_scalar(si0, si0, 0.1, 1.0, OP.mult, OP.add)
        nc.vector.tensor_scalar(st0, si0, -1.0, 2.0, OP.mult, OP.add)

        coef = singles.tile([128, RC_TOT, 6], f32, tag="coef")  # A_i B_i A_t B_t w_i w_t
        A_i = coef[:, :, 0:1].rearrange("p c 1 -> p c")
        B_i = coef[:, :, 1:2].rearrange("p c 1 -> p c")
        A_t = coef[:, :, 2:3].rearrange("p c 1 -> p c")
        B_t = coef[:, :, 3:4].rearrange("p c 1 -> p c")
        w_i = coef[:, :, 4:5].rearrange("p c 1 -> p c")
        w_t = coef[:, :, 5:6].rearrange("p c 1 -> p c")

        t_a = sc("t_a"); t_b = sc("t_b")
        # A_i = e0*si0 + e1 + e2*0.3*img_auth
        nc.vector.scalar_tensor_tensor(t_a, img_auth, 0.3, e2, OP.mult, OP.mult)
        nc.vector.tensor_mul(t_b, si0, e0)
        nc.vector.tensor_add(t_a, t_a, t_b)
        nc.vector.tensor_add(A_i, t_a, e1)
        # B_i = e2*(1-img_auth)*img_imp
        nc.vector.tensor_scalar(t_a, img_auth, -1.0, 1.0, OP.mult, OP.add)
        nc.vector.tensor_mul(t_a, t_a, img_imp)
        nc.vector.tensor_mul(B_i, t_a, e2)
        # A_t = e0*st0 + e1*0.3*text_auth + e2
        nc.vector.scalar_tensor_tensor(t_a, text_auth, 0.3, e1, OP.mult, OP.mult)
        nc.vector.tensor_mul(t_b, st0, e0)
        nc.vector.tensor_add(t_a, t_a, t_b)
        nc.vector.tensor_add(A_t, t_a, e2)
        # B_t = e1*(1-text_auth)*text_imp
        nc.vector.tensor_scalar(t_a, text_auth, -1.0, 1.0, OP.mult, OP.add)
        nc.vector.tensor_mul(t_a, t_a, text_imp)
        nc.vector.tensor_mul(B_t, t_a, e1)

        # ---------------- quality rows to transposed space (PE) ----------------
        qualT = singles.tile([11, B_CORE], bf16, tag="qualT")
        for g in range(4):
            pst = ps_tr.tile([128, 512], bf16, tag="tr", name="pstq")
            for j in range(4):
                c = 4 * g + j
                nc.tensor.transpose(pst[0:11, j * 128:(j + 1) * 128],
                                    qual_bf[:, c, :], ident)
            nc.vector.tensor_copy(qualT[:, g * 512:(g + 1) * 512], pst[0:11, :])

        # ---------------- tiny MLPs in transposed space ----------------
        q_attT = singles.tile([1, B_CORE], bf16, tag="q_attT")
        img_wT = singles.tile([1, B_CORE], bf16, tag="img_wT")
        for n in range(N_TILES):
            sl = slice(n * TILE_N, (n + 1) * TILE_N)
            ps1 = ps_mm.tile([64, TILE_N], f32, tag="mm", name="ps1")
            nc.tensor.matmul(ps1, qaw1, qualT[:, sl], start=True, stop=True)
            g1 = finp.tile([64, TILE_N], bf16, tag="qg1", name="g1")
            nc.scalar.activation(g1, ps1, AF.Gelu, bias=qab1)
            psm1 = ps_mm.tile([32, TILE_N], f32, tag="mm", name="psm1")
            nc.tensor.matmul(psm1, miw1, qualT[:, sl], start=True, stop=True)
            mg = finp.tile([32, TILE_N], bf16, tag="mg", name="mg")
            nc.scalar.activation(mg, psm1, AF.Gelu, bias=mib1)
            ps2 = ps_mm.tile([32, TILE_N], f32, tag="mm", name="ps2")
            nc.tensor.matmul(ps2, qaw2, g1, start=True, stop=True)
            g2 = finp.tile([32, TILE_N], bf16, tag="qg2", name="g2")
            nc.scalar.activation(g2, ps2, AF.Gelu, bias=qab2)
            psm2 = ps_mm.tile([1, TILE_N], f32, tag="mm", name="psm2")
            nc.tensor.matmul(psm2, wdiff, mg, start=True, stop=True)
            nc.scalar.activation(img_wT[:, sl], psm2, AF.Tanh, bias=db, scale=0.5)
            ps3 = ps_mm.tile([1, TILE_N], f32, tag="mm", name="ps3")
            nc.tensor.matmul(ps3, qaw3, g2, start=True, stop=True)
            nc.scalar.activation(q_attT[:, sl], ps3, AF.Tanh, bias=qab3h, scale=0.5)

        # gates back to row-major [128, RC_TOT, 2]
        mlprm = singles.tile([128, RC_TOT, 2], f32, tag="mlprm")
        for g in range(4):
            # bf16 PSUM writes need 4-byte alignment -> even column offsets
            pst = ps_tr.tile([128, 512], bf16, tag="tr", name="pstg")
            for j in range(4):
                c = 4 * g + j
                cs = slice(c * 128, (c + 1) * 128)
                nc.tensor.transpose(pst[:, 4 * j:4 * j + 1], q_attT[:, cs],
                                    ident[0:1, 0:1])
                nc.tensor.transpose(pst[:, 4 * j + 2:4 * j + 3], img_wT[:, cs],
                                    ident[0:1, 0:1])
            pview = pst[:, 0:16].rearrange("p (c q) -> p c q", c=4)
            nc.vector.tensor_copy(mlprm[:, 4 * g:4 * (g + 1), 0:1],
                                  pview[:, :, 0:1])
            nc.vector.tensor_copy(mlprm[:, 4 * g:4 * (g + 1), 1:2],
                                  pview[:, :, 2:3])

        q_att_rm = mlprm[:, :, 0:1].rearrange("p c 1 -> p c")
        img_w_rm = mlprm[:, :, 1:2].rearrange("p c 1 -> p c")
        # gates from tanh halves: q_att = 0.5(1+hq), img_w = 0.5(1+hw)
        # w_i = q_att*img_w = 0.25(1+hq)(1+hw) ; w_t = q_att - w_i
        nc.vector.tensor_scalar(t_b, img_w_rm, 1.0, None, OP.add)
        nc.vector.scalar_tensor_tensor(w_i, q_att_rm, 1.0, t_b, OP.add, OP.mult)
        nc.vector.tensor_scalar(w_i, w_i, 0.25, None, OP.mult)
        nc.vector.tensor_scalar(t_b, q_att_rm, 0.5, 0.5, OP.mult, OP.add)
        nc.vector.tensor_sub(w_t, t_b, w_i)

        prol.__exit__(None, None, None)

        # ---------------- main loop helpers ----------------
        fin_specs = [(0, 2, A_i, B_i), (1, 3, A_t, B_t)]

        def emit_combine_and_transpose(t, in_sb):
            """Row-major combine (vector) + xbar transpose to feature-major."""
            finT = []
            for pi, (bfi, efi, Ac, Bc) in enumerate(fin_specs):
                fT = fintp.tile([128, PC, TILE_N], bf16, tag="finT", name="fT")
                for c in range(PC):
                    g = t * PC + c
                    tmp = finp.tile([128, H], bf16, tag="ctmp", name="tmp")
                    nc.vector.tensor_scalar(tmp, in_sb[efi][:, c, :],
                                            Bc[:, g:g + 1], None, OP.mult)
                    ft = finp.tile([128, H], bf16, tag="fin", name="ft")
                    nc.vector.scalar_tensor_tensor(ft, in_sb[bfi][:, c, :],
                                                   Ac[:, g:g + 1], tmp,
                                                   OP.mult, OP.add)
                    nc.sync.dma_start(out=fT[:, :, c * 128:(c + 1) * 128],
                                      in_=ft, transpose=True)
                finT.append(fT)
            return finT

        def emit_z1(t, finT):
            tsl = slice(t * TILE_N, (t + 1) * TILE_N)
            g1T = {}
            for m in range(PC):
                ms = slice(m * 128, (m + 1) * 128)
                for pi in range(2):
                    z1 = ps_mm.tile([128, TILE_N], f32, tag="mm", name="z1")
                    nc.tensor.matmul(z1, wr1[:, ms], DballB[:, tsl],
                                     start=True, stop=False)
                    for k in range(PC):
                        nc.tensor.matmul(z1, dcw1[:, k, ms], finT[pi][:, k, :],
                                         start=False, stop=(k == PC - 1))
                    gt = g1p.tile([128, TILE_N], bf16, tag="g1", name="gt")
                    nc.scalar.activation(gt, z1, AF.Gelu, bias=dcb1[:, m:m + 1])
                    g1T[(pi, m)] = gt
            return g1T

        def emit_z2(t, g1T):
            stT = {}
            for m in range(PC):
                ms = slice(m * 128, (m + 1) * 128)
                for pi in range(2):
                    z2 = ps_mm.tile([128, TILE_N], f32, tag="mm", name="z2")
                    for k in range(PC):
                        nc.tensor.matmul(z2, dcw2[:, k, ms], g1T[(pi, k)],
                                         start=(k == 0), stop=(k == PC - 1))
                    st = stp.tile([128, TILE_N], bf16, tag="sT", name="st")
                    nc.scalar.activation(st, z2, AF.Tanh, bias=dcb2h[:, m:m + 1],
                                         scale=0.5)
                    stT[(pi, m)] = st
            return stT

        def emit_comp(t, finT, stT):
            tsl = slice(t * TILE_N, (t + 1) * TILE_N)
            compT = {}
            for pi in range(2):
                for m in range(PC):
                    t1 = t1p.tile([128, TILE_N], bf16, tag="t1", name="t1")
                    nc.vector.scalar_tensor_tensor(t1, stT[(pi, m)], 1.0,
                                                   DballB[:, tsl], OP.add, OP.mult)
                    ct = compp.tile([128, TILE_N], bf16, tag="comp", name="ct")
                    nc.vector.scalar_tensor_tensor(ct, t1, 1.0, finT[pi][:, m, :],
                                                   OP.add, OP.mult)
                    compT[(pi, m)] = ct
            return compT

        def emit_attention(t, compT):
            for srcp, wcol, ocol in [(0, w_t, 1), (1, w_i, 0)]:
                for r in range(PC):
                    g = t * PC + r
                    att = ps_mm.tile([128, H], f32, tag="mm", name="att")
                    for k in range(PC):
                        nc.tensor.matmul(att, compT[(srcp, k)][:, r * 128:(r + 1) * 128],
                                         wc[:, k, :], start=(k == 0),
                                         stop=(not use_bvo and k == PC - 1))
                    if use_bvo:
                        nc.tensor.matmul(att, ones_r, bvo, start=False, stop=True)
                    ot = outp.tile([128, H], f32, tag="out", name="ot")
                    nc.scalar.activation(ot, att, AF.Copy, scale=wcol[:, g:g + 1])
                    nc.sync.dma_start(
                        out=d_out[t * TILE_N + r * 128: t * TILE_N + (r + 1) * 128,
                                  ocol * H:(ocol + 1) * H],
                        in_=ot)

        # ---------------- main loop ----------------
        finT = emit_combine_and_transpose(0, in_sb0)
        for t in range(N_TILES):
            scope = nc.named_scope(f"tile{t}")
            scope.__enter__()
            g1T = emit_z1(t, finT)
            if t + 1 < N_TILES:
                in_next = emit_loads(t + 1)
            stT = emit_z2(t, g1T)
            if t + 1 < N_TILES:
                finT_next = emit_combine_and_transpose(t + 1, in_next)
            else:
                finT_next = None
            compT = emit_comp(t, finT, stT)
            emit_attention(t, compT)
            finT = finT_next
            scope.__exit__(None, None, None)

    nc.compile()
    _dedupe_ldweights(nc, mybir)
    return nc


def _dedupe_ldweights(nc, mybir):
    """Drop InstLdweights that reload the exact weights already resident in
    the PE array (no intervening loads). Only sync-free LDWs are removed."""
    removed = 0
    for blk in nc.m.functions[0].blocks:
        insts = list(blk.instructions)
        keep = []
        cur = None
        for i in insts:
            if getattr(i, 'engine', None) != mybir.EngineType.PE:
                keep.append(i)
                continue
            t = type(i).__name__
            if t == 'InstLdweights':
                ap = i.ins[0]
                key = (str(ap.memref), ap.offset, str(ap.ap), str(ap.dtype),
                       bool(getattr(i, 'is_transpose', False)),
                       str(getattr(i, 'perf_mode', None)),
                       str(getattr(i, 'tile_position', None)))
                si = i.sync_info
                has_sync = bool(si and (si.on_wait or si.on_update))
                if key == cur and not has_sync:
                    removed += 1
                    continue
                cur = key
                keep.append(i)
            elif t == 'InstMatmult':
                keep.append(i)
            else:
                cur = None
                keep.append(i)
        if removed:
            blk.instructions = keep
    return removed


def _get_program(use_bvo=False):
    key = ("nc", use_bvo)
    if key not in _CACHE:
        _CACHE[key] = _build_program(use_bvo)
    return _CACHE[key]


def kernel(**inputs) -> np.ndarray:
    global last_exec_time_ns, last_trace_path, last_scope_times
    import ml_dtypes
    from concourse.bass_utils import run_bass_kernel_spmd

    bf16 = ml_dtypes.bfloat16

    f = {k: np.ascontiguousarray(np.asarray(v, dtype=np.float32))
         for k, v in inputs.items() if k != "missing_type"}
    missing_f = np.ascontiguousarray(
        np.asarray(inputs["missing_type"]).astype(np.float32))

    # value-specialize: v/o projection biases are zero in this problem
    use_bvo = bool(np.any(f["bv"]) or np.any(f["bo"]))
    nc = _get_program(use_bvo)

    # host-side weight prep (replicated across cores)
    wc = (f["wv"] @ f["wo"]).astype(bf16)
    miw1p = np.zeros((11, 32), np.float32)
    miw1p[6:10] = f["mi_w1"]
    weights = {
        "dcw1b": f["dc_w1"][:H].astype(bf16),
        "wr1b": np.ascontiguousarray(
            np.broadcast_to(f["dc_w1"][H] * (2.0 / 128.0), (128, H))).astype(bf16),
        "dc_b1": f["dc_b1"],
        "dcw2b": f["dc_w2"].astype(bf16),
        "dcb2h": 0.5 * f["dc_b2"],
        "wcb": wc,
        "qa_w1": f["qa_w1"].astype(bf16),
        "qa_b1": f["qa_b1"],
        "qa_w2": f["qa_w2"].astype(bf16),
        "qa_b2": f["qa_b2"],
        "qa_w3": f["qa_w3"].astype(bf16),
        "qab3h": 0.5 * f["qa_b3"],
        "miw1p": miw1p.astype(bf16),
        "mi_b1": f["mi_b1"],
        "wdiff": np.ascontiguousarray(f["mi_w2"][:, 0:1] - f["mi_w2"][:, 1:2]).astype(bf16),
        "db": 0.5 * (f["mi_b2"][0:1] - f["mi_b2"][1:2]),
    }
    if use_bvo:
        weights["bvo"] = (f["bv"] @ f["wo"] + f["bo"]).astype(bf16)

    feats_bf = {k: f[k].astype(bf16) for k in
                ["image_feat", "text_feat", "enhanced_image_feat",
                 "enhanced_text_feat"]}

    in_maps = []
    for c in range(N_CORES):
        sl = slice(c * B_CORE, (c + 1) * B_CORE)
        m = {k: v[sl] for k, v in feats_bf.items()}
        m["quality"] = f["quality"][sl]
        m["missing_f"] = missing_f[sl]
        m.update(weights)
        in_maps.append(m)

    trace = os.environ.get("KERNEL_TRACE", "0") == "1"
    res = run_bass_kernel_spmd(nc, in_maps, core_ids=list(range(N_CORES)),
                               trace=trace)
    last_exec_time_ns = res.exec_time_ns
    last_scope_times = res.per_core_scope_times
    if res.instructions_and_trace is not None:
        last_trace_path = res.instructions_and_trace[1]

    out = np.empty((B_FULL, 2 * H), dtype=np.float32)
    for c in range(N_CORES):
        out[c * B_CORE:(c + 1) * B_CORE] = res.results[c]["out"]
    return out


# revision 13
# speedup vs baseline: 1.2139x; 1.2139x over previous
"""Trainium2 Bass kernel for nn_AttentionReweightingFusion.

Contract: kernel(**inputs) takes FULL (unsharded) numpy inputs as produced by
setup_inputs() and returns the FULL [16384, 1024] float32 output.

Strategy (pure data parallel over 8 NeuronCores, weights replicated):
  - 2048 batch rows per core, processed in 4 tiles of 512 rows.
  - Features and big weights are pre-cast to bf16 on the host, halving HBM
    input traffic; wv@wo is collapsed host-side into a single wc matrix.
  - Per-row scalar math (missing-type coefficients, thresholds) runs in exact
    fp32 in row-major layout where per-row values are native [128,1] scalars.
  - Row-major -> feature-major conversion of the combined features uses the
    xbar DMA transpose (one call turns a [128 rows, 512 feat] chunk into the
    [feat, chunk, row] layout the matmuls consume), keeping the PE stream
    free for real matmuls.
  - The difficulty column's rank-1 contribution to the first dc matmul is
    computed as a normal K=128 matmul against a broadcast difficulty tile
    using a host-prepared weight spread over 128 partitions.
  - seq_len==kv_len==1 MHA reduces to out_proj(v_proj(x)) == x @ wc.
"""

import os

import numpy as np

H = 512
B_FULL = 16384
N_CORES = 8
B_CORE = B_FULL // N_CORES          # 2048
TILE_N = 512                        # batch rows per compute tile
N_TILES = B_CORE // TILE_N          # 4
PC = H // 128                       # feature chunks of 128 (4)
RC_TOT = B_CORE // 128              # row chunks per core (16)

_CACHE: dict = {}

# Exposed for test.py after a profiled run
last_exec_time_ns = None
last_trace_path = None
last_scope_times = None


def _build_program(use_bvo=False):
    from contextlib import ExitStack

    import concourse.bacc as bacc
    import concourse.mybir as mybir
    import concourse.tile as tile
    from concourse.masks import make_identity

    dt = mybir.dt
    f32 = dt.float32
    bf16 = dt.bfloat16
    AF = mybir.ActivationFunctionType
    OP = mybir.AluOpType

    nc = bacc.Bacc(num_swdge_queues=4)

    # ---------------- DRAM I/O (per-core shapes) ----------------
    d_img = nc.dram_tensor("image_feat", [B_CORE, H], bf16, kind="ExternalInput")
    d_txt = nc.dram_tensor("text_feat", [B_CORE, H], bf16, kind="ExternalInput")
    d_eimg = nc.dram_tensor("enhanced_image_feat", [B_CORE, H], bf16, kind="ExternalInput")
    d_etxt = nc.dram_tensor("enhanced_text_feat", [B_CORE, H], bf16, kind="ExternalInput")
    d_qual = nc.dram_tensor("quality", [B_CORE, 11], f32, kind="ExternalInput")
    d_miss = nc.dram_tensor("missing_f", [B_CORE], f32, kind="ExternalInput")

    d_dcw1 = nc.dram_tensor("dcw1b", [H, H], bf16, kind="ExternalInput")
    d_wr1 = nc.dram_tensor("wr1b", [128, H], bf16, kind="ExternalInput")
    d_dcb1 = nc.dram_tensor("dc_b1", [H], f32, kind="ExternalInput")
    d_dcw2 = nc.dram_tensor("dcw2b", [H, H], bf16, kind="ExternalInput")
    d_dcb2h = nc.dram_tensor("dcb2h", [H], f32, kind="ExternalInput")
    d_wc = nc.dram_tensor("wcb", [H, H], bf16, kind="ExternalInput")

    d_qa_w1 = nc.dram_tensor("qa_w1", [11, 64], bf16, kind="ExternalInput")
    d_qa_b1 = nc.dram_tensor("qa_b1", [64], f32, kind="ExternalInput")
    d_qa_w2 = nc.dram_tensor("qa_w2", [64, 32], bf16, kind="ExternalInput")
    d_qa_b2 = nc.dram_tensor("qa_b2", [32], f32, kind="ExternalInput")
    d_qa_w3 = nc.dram_tensor("qa_w3", [32, 1], bf16, kind="ExternalInput")
    d_qab3h = nc.dram_tensor("qab3h", [1], f32, kind="ExternalInput")
    d_miw1p = nc.dram_tensor("miw1p", [11, 32], bf16, kind="ExternalInput")
    d_mi_b1 = nc.dram_tensor("mi_b1", [32], f32, kind="ExternalInput")
    d_wdiff = nc.dram_tensor("wdiff", [32, 1], bf16, kind="ExternalInput")
    d_db = nc.dram_tensor("db", [1], f32, kind="ExternalInput")
    d_seld = nc.dram_tensor("seld", [11, 1], bf16, kind="ExternalInput")
    if use_bvo:
        d_bvo = nc.dram_tensor("bvo", [H], bf16, kind="ExternalInput")

    d_out = nc.dram_tensor("out", [B_CORE, 2 * H], f32, kind="ExternalOutput")

    with tile.TileContext(nc) as tc, ExitStack() as ctx:
        singles = ctx.enter_context(tc.tile_pool(name="singles", bufs=1))
        inp = ctx.enter_context(tc.tile_pool(name="inp", bufs=6))
        ps_mm = ctx.enter_context(tc.tile_pool(name="ps_mm", bufs=6, space="PSUM"))
        ps_tr = ctx.enter_context(tc.tile_pool(name="ps_tr", bufs=2, space="PSUM"))
        finp = ctx.enter_context(tc.tile_pool(name="finp", bufs=12))
        fintp = ctx.enter_context(tc.tile_pool(name="fintp", bufs=4))
        g1p = ctx.enter_context(tc.tile_pool(name="g1p", bufs=10))
        stp = ctx.enter_context(tc.tile_pool(name="stp", bufs=9))
        t1p = ctx.enter_context(tc.tile_pool(name="t1p", bufs=3))
        compp = ctx.enter_context(tc.tile_pool(name="compp", bufs=9))
        outp = ctx.enter_context(tc.tile_pool(name="outp", bufs=6))

        feats = [d_img, d_txt, d_eimg, d_etxt]

        # ---------------- async DMA loads ----------------
        # sync (HWDGE) queue: only data whose consumers come first, so the
        # FIFO never blocks feature loads behind weight transfers.
        qual = singles.tile([128, RC_TOT, 11], f32, tag="qual")
        nc.sync.dma_start(out=qual, in_=d_qual.rearrange("(c p) f -> p c f", p=128))
        mrm = singles.tile([128, RC_TOT], f32, tag="mrm")
        nc.sync.dma_start(out=mrm, in_=d_miss.rearrange("(c p) -> p c", p=128))

        # weights go through the SWDGE (gpsimd) queues, in consumer order
        qaw1 = singles.tile([11, 64], bf16, tag="qaw1")
        nc.gpsimd.dma_start(out=qaw1, in_=d_qa_w1[:, :])
        miw1 = singles.tile([11, 32], bf16, tag="miw1")
        nc.gpsimd.dma_start(out=miw1, in_=d_miw1p[:, :])
        qab1 = singles.tile([64, 1], f32, tag="qab1")
        nc.gpsimd.dma_start(out=qab1, in_=d_qa_b1[:].unsqueeze(1))
        mib1 = singles.tile([32, 1], f32, tag="mib1")
        nc.gpsimd.dma_start(out=mib1, in_=d_mi_b1[:].unsqueeze(1))
        dcw1 = singles.tile([128, PC, H], bf16, tag="dcw1")
        nc.gpsimd.dma_start(out=dcw1, in_=d_dcw1.rearrange("(c p) f -> p c f", p=128))
        wr1 = singles.tile([128, H], bf16, tag="wr1")
        nc.gpsimd.dma_start(out=wr1, in_=d_wr1[:, :])
        dcb1 = singles.tile([128, PC], f32, tag="dcb1")
        nc.gpsimd.dma_start(out=dcb1, in_=d_dcb1.rearrange("(m p) -> p m", p=128))

        def emit_loads(t):
            in_sb = []
            for dten in feats:
                it = inp.tile([128, PC, H], bf16, tag="in", name="it")
                nc.sync.dma_start(
                    out=it,
                    in_=dten[t * TILE_N:(t + 1) * TILE_N, :].rearrange(
                        "(c p) f -> p c f", p=128))
                in_sb.append(it)
            return in_sb

        in_sb0 = emit_loads(0)

        ident = singles.tile([128, 128], bf16, tag="ident")
        make_identity(nc, ident)
        # one-hot selector (value 0.5) to pull D/2 out of qualT via matmul
        sel_d = singles.tile([11, 1], bf16, tag="sel_d")
        nc.gpsimd.dma_start(out=sel_d, in_=d_seld[:, :])

        prol = nc.named_scope("prol")
        prol.__enter__()

        # ---------------- exact fp32 per-row coefficient math ----------------
        qual_bf = singles.tile([128, RC_TOT, 11], bf16, tag="qual_bf")
        nc.vector.tensor_copy(qual_bf, qual)

        def sc(tag):
            return singles.tile([128, RC_TOT], f32, tag=tag, name=tag)

        img_imp = qual[:, :, 6:7].rearrange("p c 1 -> p c")
        text_imp = qual[:, :, 7:8].rearrange("p c 1 -> p c")
        img_auth = qual[:, :, 8:9].rearrange("p c 1 -> p c")
        text_auth = qual[:, :, 9:10].rearrange("p c 1 -> p c")

        e0 = sc("e0"); e1 = sc("e1"); e2 = sc("e2")
        nc.vector.tensor_scalar(e0, mrm, 0.5, None, OP.is_lt)
        nc.vector.tensor_scalar(e1, mrm, 1.0, None, OP.is_equal)
        nc.vector.tensor_scalar(e2, mrm, 1.5, None, OP.is_gt)

        den = sc("den"); ratio = sc("ratio")
        nc.vector.scalar_tensor_tensor(den, img_imp, 1e-8, text_imp, OP.add, OP.add)
        nc.vector.reciprocal(den, den)
        nc.vector.tensor_mul(ratio, img_imp, den)
        ghi = sc("ghi"); glo = sc("glo"); si0 = sc("si0"); st0 = sc("st0")
        nc.vector.tensor_scalar(ghi, ratio, 0.6, None, OP.is_gt)
        nc.vector.tensor_scalar(glo, ratio, 0.4, None, OP.is_lt)
        nc.vector.tensor_sub(si0, ghi, glo)
        nc.vector.tensor_scalar(si0, si0, 0.1, 1.0, OP.mult, OP.add)
        nc.vector.tensor_scalar(st0, si0, -1.0, 2.0, OP.mult, OP.add)

        coef = singles.tile([128, RC_TOT, 6], f32, tag="coef")  # A_i B_i A_t B_t w_i w_t
        A_i = coef[:, :, 0:1].rearrange("p c 1 -> p c")
        B_i = coef[:, :, 1:2].rearrange("p c 1 -> p c")
        A_t = coef[:, :, 2:3].rearrange("p c 1 -> p c")
        B_t = coef[:, :, 3:4].rearrange("p c 1 -> p c")
        w_i = coef[:, :, 4:5].rearrange("p c 1 -> p c")
        w_t = coef[:, :, 5:6].rearrange("p c 1 -> p c")

        t_a = sc("t_a"); t_b = sc("t_b")
        # A_i = e0*si0 + e1 + e2*0.3*img_auth
        nc.vector.scalar_tensor_tensor(t_a, img_auth, 0.3, e2, OP.mult, OP.mult)
        nc.vector.tensor_mul(t_b, si0, e0)
        nc.vector.tensor_add(t_a, t_a, t_b)
        nc.vector.tensor_add(A_i, t_a, e1)
        # B_i = e2*(1-img_auth)*img_imp
        nc.vector.tensor_scalar(t_a, img_auth, -1.0, 1.0, OP.mult, OP.add)
        nc.vector.tensor_mul(t_a, t_a, img_imp)
        nc.vector.tensor_mul(B_i, t_a, e2)
        # A_t = e0*st0 + e1*0.3*text_auth + e2
        nc.vector.scalar_tensor_tensor(t_a, text_auth, 0.3, e1, OP.mult, OP.mult)
        nc.vector.tensor_mul(t_b, st0, e0)
        nc.vector.tensor_add(t_a, t_a, t_b)
        nc.vector.tensor_add(A_t, t_a, e2)
        # B_t = e1*(1-text_auth)*text_imp
        nc.vector.tensor_scalar(t_a, text_auth, -1.0, 1.0, OP.mult, OP.add)
        nc.vector.tensor_mul(t_a, t_a, text_imp)
        nc.vector.tensor_mul(B_t, t_a, e1)

        # ---------------- quality rows to transposed space (PE) ----------------
        qualT = singles.tile([11, B_CORE], bf16, tag="qualT")
        for g in range(4):
            pst = ps_tr.tile([128, 512], bf16, tag="tr", name="pstq")
            for j in range(4):
                c = 4 * g + j
                nc.tensor.transpose(pst[0:11, j * 128:(j + 1) * 128],
                                    qual_bf[:, c, :], ident)
            nc.vector.tensor_copy(qualT[:, g * 512:(g + 1) * 512], pst[0:11, :])

        # difficulty/2 as a [1, B] row via one-hot matmul, then broadcast
        dT_h = singles.tile([1, B_CORE], f32, tag="dT_h")
        for n in range(N_TILES):
            sl = slice(n * TILE_N, (n + 1) * TILE_N)
            psd = ps_mm.tile([1, TILE_N], f32, tag="mm", name="psd")
            nc.tensor.matmul(psd, sel_d, qualT[:, sl], start=True, stop=True)
            nc.vector.tensor_copy(dT_h[:, sl], psd)
        Dball = singles.tile([128, B_CORE], f32, tag="Dball")
        nc.gpsimd.partition_broadcast(Dball, dT_h)
        DballB = singles.tile([128, B_CORE], bf16, tag="DballB")   # D/2 bf16
        nc.vector.tensor_copy(DballB, Dball)

        # remaining weights (SWDGE queues), in consumer order
        qaw2 = singles.tile([64, 32], bf16, tag="qaw2")
        nc.gpsimd.dma_start(out=qaw2, in_=d_qa_w2[:, :])
        qab2 = singles.tile([32, 1], f32, tag="qab2")
        nc.gpsimd.dma_start(out=qab2, in_=d_qa_b2[:].unsqueeze(1))
        qaw3 = singles.tile([32, 1], bf16, tag="qaw3")
        nc.gpsimd.dma_start(out=qaw3, in_=d_qa_w3[:, :])
        qab3h = singles.tile([1, 1], f32, tag="qab3h")
        nc.gpsimd.dma_start(out=qab3h, in_=d_qab3h[:].unsqueeze(1))
        wdiff = singles.tile([32, 1], bf16, tag="wdiff")
        nc.gpsimd.dma_start(out=wdiff, in_=d_wdiff[:, :])
        db = singles.tile([1, 1], f32, tag="db")
        nc.gpsimd.dma_start(out=db, in_=d_db[:].unsqueeze(1))
        dcb2h = singles.tile([128, PC], f32, tag="dcb2h")
        nc.gpsimd.dma_start(out=dcb2h, in_=d_dcb2h.rearrange("(m p) -> p m", p=128))
        dcw2 = singles.tile([128, PC, H], bf16, tag="dcw2")
        nc.gpsimd.dma_start(out=dcw2, in_=d_dcw2.rearrange("(c p) f -> p c f", p=128))
        wc = singles.tile([128, PC, H], bf16, tag="wc")
        nc.gpsimd.dma_start(out=wc, in_=d_wc.rearrange("(c p) f -> p c f", p=128))
        if use_bvo:
            bvo = singles.tile([1, H], bf16, tag="bvo")
            nc.gpsimd.dma_start(out=bvo, in_=d_bvo[:].unsqueeze(0))
            ones_r = singles.tile([1, 128], bf16, tag="ones_r")
            nc.vector.memset(ones_r, 1.0)

        # combine+transpose for tile 0 ahead of the tiny MLPs so the PE's
        # first z-chain has its inputs as early as possible
        fin_specs = [(0, 2, A_i, B_i), (1, 3, A_t, B_t)]

        def emit_combine_and_transpose(t, in_sb):
            """Row-major combine (vector) + xbar transpose to feature-major."""
            finT = []
            for pi, (bfi, efi, Ac, Bc) in enumerate(fin_specs):
                fT = fintp.tile([128, PC, TILE_N], bf16, tag="finT", name="fT")
                for c in range(PC):
                    g = t * PC + c
                    tmp = finp.tile([128, H], bf16, tag="ctmp", name="tmp")
                    nc.vector.tensor_scalar(tmp, in_sb[efi][:, c, :],
                                            Bc[:, g:g + 1], None, OP.mult)
                    ft = finp.tile([128, H], bf16, tag="fin", name="ft")
                    nc.vector.scalar_tensor_tensor(ft, in_sb[bfi][:, c, :],
                                                   Ac[:, g:g + 1], tmp,
                                                   OP.mult, OP.add)
                    nc.sync.dma_start(out=fT[:, :, c * 128:(c + 1) * 128],
                                      in_=ft, transpose=True)
                finT.append(fT)
            return finT

        finT0 = emit_combine_and_transpose(0, in_sb0)

        # ---------------- tiny MLPs in transposed space ----------------
        q_attT = singles.tile([1, B_CORE], bf16, tag="q_attT")
        img_wT = singles.tile([1, B_CORE], bf16, tag="img_wT")
        for n in range(N_TILES):
            sl = slice(n * TILE_N, (n + 1) * TILE_N)
            ps1 = ps_mm.tile([64, TILE_N], f32, tag="mm", name="ps1")
            nc.tensor.matmul(ps1, qaw1, qualT[:, sl], start=True, stop=True)
            g1 = finp.tile([64, TILE_N], bf16, tag="qg1", name="g1")
            nc.scalar.activation(g1, ps1, AF.Gelu, bias=qab1)
            psm1 = ps_mm.tile([32, TILE_N], f32, tag="mm", name="psm1")
            nc.tensor.matmul(psm1, miw1, qualT[:, sl], start=True, stop=True)
            mg = finp.tile([32, TILE_N], bf16, tag="mg", name="mg")
            nc.scalar.activation(mg, psm1, AF.Gelu, bias=mib1)
            ps2 = ps_mm.tile([32, TILE_N], f32, tag="mm", name="ps2")
            nc.tensor.matmul(ps2, qaw2, g1, start=True, stop=True)
            g2 = finp.tile([32, TILE_N], bf16, tag="qg2", name="g2")
            nc.scalar.activation(g2, ps2, AF.Gelu, bias=qab2)
            psm2 = ps_mm.tile([1, TILE_N], f32, tag="mm", name="psm2")
            nc.tensor.matmul(psm2, wdiff, mg, start=True, stop=True)
            nc.scalar.activation(img_wT[:, sl], psm2, AF.Tanh, bias=db, scale=0.5)
            ps3 = ps_mm.tile([1, TILE_N], f32, tag="mm", name="ps3")
            nc.tensor.matmul(ps3, qaw3, g2, start=True, stop=True)
            nc.scalar.activation(q_attT[:, sl], ps3, AF.Tanh, bias=qab3h, scale=0.5)

        # gates back to row-major [128, RC_TOT, 2]
        mlprm = singles.tile([128, RC_TOT, 2], f32, tag="mlprm")
        for g in range(4):
            # bf16 PSUM writes need 4-byte alignment -> even column offsets
            pst = ps_tr.tile([128, 512], bf16, tag="tr", name="pstg")
            for j in range(4):
                c = 4 * g + j
                cs = slice(c * 128, (c + 1) * 128)
                nc.tensor.transpose(pst[:, 4 * j:4 * j + 1], q_attT[:, cs],
                                    ident[0:1, 0:1])
                nc.tensor.transpose(pst[:, 4 * j + 2:4 * j + 3], img_wT[:, cs],
                                    ident[0:1, 0:1])
            pview = pst[:, 0:16].rearrange("p (c q) -> p c q", c=4)
            nc.vector.tensor_copy(mlprm[:, 4 * g:4 * (g + 1), 0:1],
                                  pview[:, :, 0:1])
            nc.vector.tensor_copy(mlprm[:, 4 * g:4 * (g + 1), 1:2],
                                  pview[:, :, 2:3])

        q_att_rm = mlprm[:, :, 0:1].rearrange("p c 1 -> p c")
        img_w_rm = mlprm[:, :, 1:2].rearrange("p c 1 -> p c")
        # gates from tanh halves: q_att = 0.5(1+hq), img_w = 0.5(1+hw)
        # w_i = q_att*img_w = 0.25(1+hq)(1+hw) ; w_t = q_att - w_i
        nc.vector.tensor_scalar(t_b, img_w_rm, 1.0, None, OP.add)
        nc.vector.scalar_tensor_tensor(w_i, q_att_rm, 1.0, t_b, OP.add, OP.mult)
        nc.vector.tensor_scalar(w_i, w_i, 0.25, None, OP.mult)
        nc.vector.tensor_scalar(t_b, q_att_rm, 0.5, 0.5, OP.mult, OP.add)
        nc.vector.tensor_sub(w_t, t_b, w_i)

        prol.__exit__(None, None, None)

        # ---------------- main loop helpers ----------------
        def emit_z1(t, finT):
            tsl = slice(t * TILE_N, (t + 1) * TILE_N)
            g1T = {}
            for m in range(PC):
                ms = slice(m * 128, (m + 1) * 128)
                for pi in range(2):
                    z1 = ps_mm.tile([128, TILE_N], f32, tag="mm", name="z1")
                    for k in range(PC):
                        nc.tensor.matmul(z1, dcw1[:, k, ms], finT[pi][:, k, :],
                                         start=(k == 0), stop=False)
                    nc.tensor.matmul(z1, wr1[:, ms], DballB[:, tsl],
                                     start=False, stop=True)
                    gt = g1p.tile([128, TILE_N], bf16, tag="g1", name="gt")
                    nc.scalar.activation(gt, z1, AF.Gelu, bias=dcb1[:, m:m + 1])
                    g1T[(pi, m)] = gt
            return g1T

        def emit_z2(t, g1T):
            stT = {}
            for m in range(PC):
                ms = slice(m * 128, (m + 1) * 128)
                for pi in range(2):
                    z2 = ps_mm.tile([128, TILE_N], f32, tag="mm", name="z2")
                    for k in range(PC):
                        nc.tensor.matmul(z2, dcw2[:, k, ms], g1T[(pi, k)],
                                         start=(k == 0), stop=(k == PC - 1))
                    st = stp.tile([128, TILE_N], bf16, tag="sT", name="st")
                    nc.scalar.activation(st, z2, AF.Tanh, bias=dcb2h[:, m:m + 1],
                                         scale=0.5)
                    stT[(pi, m)] = st
            return stT

        def emit_comp(t, finT, stT):
            tsl = slice(t * TILE_N, (t + 1) * TILE_N)
            compT = {}
            for pi in range(2):
                for m in range(PC):
                    t1 = t1p.tile([128, TILE_N], bf16, tag="t1", name="t1")
                    nc.vector.scalar_tensor_tensor(t1, stT[(pi, m)], 1.0,
                                                   DballB[:, tsl], OP.add, OP.mult)
                    ct = compp.tile([128, TILE_N], bf16, tag="comp", name="ct")
                    nc.vector.scalar_tensor_tensor(ct, t1, 1.0, finT[pi][:, m, :],
                                                   OP.add, OP.mult)
                    compT[(pi, m)] = ct
            return compT

        def emit_attention(t, compT):
            for srcp, wcol, ocol in [(0, w_t, 1), (1, w_i, 0)]:
                for r in range(PC):
                    g = t * PC + r
                    att = ps_mm.tile([128, H], f32, tag="mm", name="att")
                    for k in range(PC):
                        nc.tensor.matmul(att, compT[(srcp, k)][:, r * 128:(r + 1) * 128],
                                         wc[:, k, :], start=(k == 0),
                                         stop=(not use_bvo and k == PC - 1))
                    if use_bvo:
                        nc.tensor.matmul(att, ones_r, bvo, start=False, stop=True)
                    ot = outp.tile([128, H], f32, tag="out", name="ot")
                    nc.scalar.activation(ot, att, AF.Copy, scale=wcol[:, g:g + 1])
                    nc.sync.dma_start(
                        out=d_out[t * TILE_N + r * 128: t * TILE_N + (r + 1) * 128,
                                  ocol * H:(ocol + 1) * H],
                        in_=ot)

        # ---------------- main loop ----------------
        finT = finT0
        for t in range(N_TILES):
            scope = nc.named_scope(f"tile{t}")
            scope.__enter__()
            g1T = emit_z1(t, finT)
            if t + 1 < N_TILES:
                in_next = emit_loads(t + 1)
            stT = emit_z2(t, g1T)
            if t + 1 < N_TILES:
                finT_next = emit_combine_and_transpose(t + 1, in_next)
            else:
                finT_next = None
            compT = emit_comp(t, finT, stT)
            emit_attention(t, compT)
            finT = finT_next
            scope.__exit__(None, None, None)

    nc.compile()
    _dedupe_ldweights(nc, mybir)
    return nc


def _dedupe_ldweights(nc, mybir):
    """Drop InstLdweights that reload the exact weights already resident in
    the PE array (no intervening loads). Only sync-free LDWs are removed."""
    removed = 0
    for blk in nc.m.functions[0].blocks:
        insts = list(blk.instructions)
        keep = []
        cur = None
        for i in insts:
            if getattr(i, 'engine', None) != mybir.EngineType.PE:
                keep.append(i)
                continue
            t = type(i).__name__
            if t == 'InstLdweights':
                ap = i.ins[0]
                key = (str(ap.memref), ap.offset, str(ap.ap), str(ap.dtype),
                       bool(getattr(i, 'is_transpose', False)),
                       str(getattr(i, 'perf_mode', None)),
                       str(getattr(i, 'tile_position', None)))
                si = i.sync_info
                has_sync = bool(si and (si.on_wait or si.on_update))
                if key == cur and not has_sync:
                    removed += 1
                    continue
                cur = key
                keep.append(i)
            elif t == 'InstMatmult':
                keep.append(i)
            else:
                cur = None
                keep.append(i)
        if removed:
            blk.instructions = keep
    return removed


def _get_program(use_bvo=False):
    key = ("nc", use_bvo)
    if key not in _CACHE:
        _CACHE[key] = _build_program(use_bvo)
    return _CACHE[key]


def kernel(**inputs) -> np.ndarray:
    global last_exec_time_ns, last_trace_path, last_scope_times
    import ml_dtypes
    from concourse.bass_utils import run_bass_kernel_spmd

    bf16 = ml_dtypes.bfloat16

    f = {k: np.ascontiguousarray(np.asarray(v, dtype=np.float32))
         for k, v in inputs.items() if k != "missing_type"}
    missing_f = np.ascontiguousarray(
        np.asarray(inputs["missing_type"]).astype(np.float32))

    # value-specialize: v/o projection biases are zero in this problem
    use_bvo = bool(np.any(f["bv"]) or np.any(f["bo"]))
    nc = _get_program(use_bvo)

    # host-side weight prep (replicated across cores)
    wc = (f["wv"] @ f["wo"]).astype(bf16)
    miw1p = np.zeros((11, 32), np.float32)
    miw1p[6:10] = f["mi_w1"]
    weights = {
        "dcw1b": f["dc_w1"][:H].astype(bf16),
        "wr1b": np.ascontiguousarray(
            np.broadcast_to(f["dc_w1"][H] * (2.0 / 128.0), (128, H))).astype(bf16),
        "dc_b1": f["dc_b1"],
        "dcw2b": f["dc_w2"].astype(bf16),
        "dcb2h": 0.5 * f["dc_b2"],
        "wcb": wc,
        "qa_w1": f["qa_w1"].astype(bf16),
        "qa_b1": f["qa_b1"],
        "qa_w2": f["qa_w2"].astype(bf16),
        "qa_b2": f["qa_b2"],
        "qa_w3": f["qa_w3"].astype(bf16),
        "qab3h": 0.5 * f["qa_b3"],
        "miw1p": miw1p.astype(bf16),
        "mi_b1": f["mi_b1"],
        "wdiff": np.ascontiguousarray(f["mi_w2"][:, 0:1] - f["mi_w2"][:, 1:2]).astype(bf16),
        "db": 0.5 * (f["mi_b2"][0:1] - f["mi_b2"][1:2]),
        "seld": np.array([[0.0]] * 10 + [[0.5]], np.float32).astype(bf16),
    }
    if use_bvo:
        weights["bvo"] = (f["bv"] @ f["wo"] + f["bo"]).astype(bf16)

    feats_bf = {k: f[k].astype(bf16) for k in
                ["image_feat", "text_feat", "enhanced_image_feat",
                 "enhanced_text_feat"]}

    in_maps = []
    for c in range(N_CORES):
        sl = slice(c * B_CORE, (c + 1) * B_CORE)
        m = {k: v[sl] for k, v in feats_bf.items()}
        m["quality"] = f["quality"][sl]
        m["missing_f"] = missing_f[sl]
        m.update(weights)
        in_maps.append(m)

    trace = os.environ.get("KERNEL_TRACE", "0") == "1"
    res = run_bass_kernel_spmd(nc, in_maps, core_ids=list(range(N_CORES)),
                               trace=trace)
    last_exec_time_ns = res.exec_time_ns
    last_scope_times = res.per_core_scope_times
    if res.instructions_and_trace is not None:
        last_trace_path = res.instructions_and_trace[1]

    out = np.empty((B_FULL, 2 * H), dtype=np.float32)
    for c in range(N_CORES):
        out[c * B_CORE:(c + 1) * B_CORE] = res.results[c]["out"]
    return out
